# revision 1
# baseline (speedup 1.0000x reference)
"""Trainium2 Bass kernel for nn_CrossAttention (single-CLS-query cross attention).

Reference computes, per batch b:
    q = x[b,0,:] @ wq.T                  (single CLS query)
    k = x[b] @ wk.T ; v = x[b] @ wv.T
    out = softmax(q k^T / sqrt(d)) v ; y = out @ wp.T + bp

Because there is a single query token, the huge K/V projections can be
eliminated algebraically:
    scores[b,h,n] = M[b,h,:] . x[b,n,:]   with  M[b,h,:] = (SCALE*q_h) @ Wk_h
    U[b,h,:]     = sum_n attn[b,h,n] x[b,n,:]
    y[b]         = concat_h(U[b,h,:] @ Wv_h.T) @ wp.T + bp
which needs only two streaming passes over x (~2.5 GMAC total) instead of
the 155 GFLOP dense projections.

Distribution: pure data parallel over batch B=32 across 8 cores (4 batches
per core), no collectives.  Each core streams its x shard twice: once in
[C, N] layout (scores, contraction over C) and once in [N, C] layout
(weighted sum, contraction over N), since the PE can only contract over the
partition dimension.  Both layouts are prepared host-side.
"""

import numpy as np

import concourse.bass as bass
import concourse.tile as tile
from concourse import bacc, mybir
from concourse.bass_utils import run_bass_kernel_spmd

# Problem constants (hardcoded per the harness contract).
B, N, C = 32, 4096, 768
H, D = 12, 64
SCALE = D ** -0.5
NCORES = 8
BSH = B // NCORES  # batches per core

F32 = mybir.dt.float32
F32R = mybir.dt.float32r

# Phase dtype knobs.  float32r runs the PE at 1 cyc/row (vs 4 for float32)
# at reduced internal precision; float32 is the conservative choice.
C_DT = F32R  # dtype for the weighted-sum (phase C) matmuls
A_DT = F32R  # dtype for the scores (phase A) matmuls
NCHUNK = C // 128  # 6
DEBUG = False


def build_kernel():
    nc = bacc.Bacc("TRN2", target_bir_lowering=False, debug=False,
                   num_devices=NCORES)

    xT = nc.dram_tensor("xT", [BSH, C, N], A_DT, kind="ExternalInput")
    x = nc.dram_tensor("x", [BSH, N, C], C_DT, kind="ExternalInput")
    x0T = nc.dram_tensor("x0T", [C, BSH], F32, kind="ExternalInput")
    wqT = nc.dram_tensor("wqT", [C, C], F32, kind="ExternalInput")
    wk = nc.dram_tensor("wk", [C, C], F32, kind="ExternalInput")
    wvT = nc.dram_tensor("wvT", [C, C], F32, kind="ExternalInput")
    wpT = nc.dram_tensor("wpT", [C, C], F32, kind="ExternalInput")
    bp = nc.dram_tensor("bp", [1, C], F32, kind="ExternalInput")
    i12 = nc.dram_tensor("i12", [H, H], F32, kind="ExternalInput")
    y = nc.dram_tensor("y", [BSH, C], F32, kind="ExternalOutput")
    dbg = {}
    if DEBUG:
        dbg["qT"] = nc.dram_tensor("dbg_qT", [128, NCHUNK, BSH], F32,
                                   kind="ExternalOutput").ap()
        dbg["mT"] = nc.dram_tensor("dbg_mT", [128, NCHUNK, BSH, H], F32,
                                   kind="ExternalOutput").ap()
        dbg["attnT"] = nc.dram_tensor("dbg_attnT", [128, N // 128, H], F32,
                                      kind="ExternalOutput").ap()
        dbg["U"] = nc.dram_tensor("dbg_U", [H, C], F32,
                                  kind="ExternalOutput").ap()

    with tile.TileContext(nc) as tc:
        cross_attn_kernel(tc, y.ap(), xT.ap(), x.ap(), x0T.ap(), wqT.ap(),
                          wk.ap(), wvT.ap(), wpT.ap(), bp.ap(), i12.ap(), dbg)
    nc.compile()
    return nc


def cross_attn_kernel(tc, y, xT, x, x0T, wqT, wk, wvT, wpT, bp, i12, dbg={}):
    from contextlib import ExitStack
    ctx = ExitStack()
    nc = tc.nc
    with ctx:
        consts = ctx.enter_context(tc.tile_pool(name="consts", bufs=1))
        xa_pool = ctx.enter_context(tc.tile_pool(name="xa", bufs=20))
        xc_pool = ctx.enter_context(tc.tile_pool(name="xc", bufs=12))
        attn_pool = ctx.enter_context(tc.tile_pool(name="attn", bufs=2))
        small = ctx.enter_context(tc.tile_pool(name="small", bufs=2))
        ps_a = ctx.enter_context(tc.tile_pool(name="ps_a", bufs=2, space="PSUM"))
        ps_c = ctx.enter_context(tc.tile_pool(name="ps_c", bufs=1, space="PSUM"))
        ps_misc = ctx.enter_context(tc.tile_pool(name="ps_misc", bufs=2, space="PSUM"))

        # ---- constant loads ----
        # All on the scalar HWDGE queue so the sync queue starts streaming
        # x tiles immediately; wvT/wpT are deferred until P4 needs them.
        def load_w(ap_dram, name):
            t = consts.tile([128, NCHUNK, C], F32, tag=name)
            nc.scalar.dma_start(out=t, in_=ap_dram.rearrange("(a p) o -> p a o", p=128))
            return t

        wqT_sb = load_w(wqT, "wqT_sb")
        wk_sb = load_w(wk, "wk_sb")
        x0T_sb = consts.tile([128, NCHUNK, BSH], F32)
        nc.scalar.dma_start(out=x0T_sb, in_=x0T.rearrange("(a p) b -> p a b", p=128))
        i12_sb = consts.tile([H, H], F32)
        nc.scalar.dma_start(out=i12_sb, in_=i12)
        bp_sb = consts.tile([BSH, C], F32)
        nc.scalar.dma_start(
            out=bp_sb,
            in_=bass.AP(tensor=bp.tensor, offset=0, ap=[[0, BSH], [1, C]]),
        )
        qT_sb = consts.tile([128, NCHUNK, BSH], F32)
        # written by a casting tensor_copy from f32 PSUM, read by phase-A matmul
        mT_sb = consts.tile([128, NCHUNK, BSH, H], A_DT)

        # ---- P0a: qT[c_out, b] = wq @ (SCALE * x0^T), contraction over c_in ----
        for co in range(NCHUNK):
            ps_q = ps_misc.tile([128, BSH], F32, tag="misc")
            for ci in range(NCHUNK):
                nc.tensor.matmul(
                    ps_q,
                    lhsT=wqT_sb[:, ci, co * 128:(co + 1) * 128],
                    rhs=x0T_sb[:, ci, :],
                    start=(ci == 0), stop=(ci == NCHUNK - 1),
                )
            nc.vector.tensor_copy(qT_sb[:, co, :], ps_q)

        # ---- P0b: mT[c, b, h] = Wk_h^T @ qT_h  (contraction over d=64) ----
        for ci in range(NCHUNK):
            for h in range(H):
                po = (h % 2) * 64
                ch = h // 2
                ps_m = ps_misc.tile([128, BSH], F32, tag="misc")
                nc.tensor.matmul(
                    ps_m,
                    lhsT=wk_sb[po:po + 64, ch, ci * 128:(ci + 1) * 128],
                    rhs=qT_sb[po:po + 64, ch, :],
                    start=True, stop=True,
                )
                nc.vector.tensor_copy(mT_sb[:, ci, :, h], ps_m)

        ut_all = consts.tile([128, NCHUNK, BSH, H], F32)  # U^T[c, b, h]
        if dbg:
            nc.sync.dma_start(out=dbg["qT"], in_=qT_sb)
            nc.sync.dma_start(out=dbg["mT"], in_=mT_sb)

        # ---- per-batch main loop ----
        for b in range(BSH):
            # phase A: scores[h, n] = sum_c mT[c, h] * xT[c, n]; exp is fused
            # into the PSUM->SBUF move (no max subtraction needed: |scores|<8)
            attn = attn_pool.tile([H, N], F32, tag="attn")
            partials = small.tile([H, N // 512], F32, tag="partials")
            for nt in range(N // 512):
                xa = []
                for ci in range(NCHUNK):
                    t = xa_pool.tile([128, 512], A_DT, tag="xa")
                    nc.sync.dma_start(
                        out=t,
                        in_=xT[b, ci * 128:(ci + 1) * 128, nt * 512:(nt + 1) * 512],
                    )
                    xa.append(t)
                ps = ps_a.tile([H, 512], F32, tag="psA")
                for ci in range(NCHUNK):
                    nc.tensor.matmul(
                        ps,
                        lhsT=mT_sb[:, ci, b, :],
                        rhs=xa[ci],
                        start=(ci == 0), stop=(ci == NCHUNK - 1),
                    )
                nc.scalar.activation(
                    out=attn[:, nt * 512:(nt + 1) * 512], in_=ps,
                    func=mybir.ActivationFunctionType.Exp,
                    accum_out=partials[:, nt:nt + 1],
                )

            sums = small.tile([H, 1], F32, tag="sums")
            nc.vector.reduce_sum(sums, partials, axis=mybir.AxisListType.X)
            rsum = small.tile([H, 1], F32, tag="rsum")
            nc.vector.reciprocal(rsum, sums)

            # transpose attn -> attnT[n, h] chunks (PE transpose via identity);
            # the PSUM->SBUF copy also casts to the phase-C matmul dtype
            attnT = attn_pool.tile([128, N // 128, H], C_DT, tag="attnT")
            for nn in range(N // 128):
                ps_t = ps_a.tile([128, H], F32, tag="psAT")
                nc.tensor.transpose(
                    ps_t, in_=attn[:, nn * 128:(nn + 1) * 128], identity=i12_sb)
                nc.vector.tensor_copy(attnT[:, nn, :], ps_t)
            if dbg and b == 0:
                nc.sync.dma_start(out=dbg["attnT"], in_=attnT)

            # phase C: U[h, c] = sum_n attnT[n, h] * x[n, c]
            psU0 = ps_c.tile([H, 384], F32, tag="psC0")
            psU1 = ps_c.tile([H, 384], F32, tag="psC1")
            psU = [psU0, psU1]
            for nn in range(N // 128):
                xc = xc_pool.tile([128, C], C_DT, tag="xc")
                # issue phase-C loads on the other HWDGE engine so the two
                # x streams ride independent DMA queues
                nc.scalar.dma_start(out=xc, in_=x[b, nn * 128:(nn + 1) * 128, :])
                for j in range(2):
                    nc.tensor.matmul(
                        psU[j],
                        lhsT=attnT[:, nn, :],
                        rhs=xc[:, j * 384:(j + 1) * 384],
                        start=(nn == 0), stop=(nn == N // 128 - 1),
                    )
            # normalize by softmax sum while moving PSUM -> SBUF
            U_sb = small.tile([H, C], F32, tag="U")
            for j in range(2):
                nc.vector.tensor_scalar_mul(
                    out=U_sb[:, j * 384:(j + 1) * 384], in0=psU[j], scalar1=rsum,
                )

            if dbg and b == 0:
                nc.sync.dma_start(out=dbg["U"], in_=U_sb)
            # transpose U -> UT[c, h] chunks for the output projections
            for k in range(NCHUNK):
                ps_t = ps_misc.tile([128, H], F32, tag="misc")
                nc.tensor.transpose(ps_t, in_=U_sb[:, k * 128:(k + 1) * 128],
                                    identity=i12_sb)
                nc.vector.tensor_copy(ut_all[:, k, b, :], ps_t)

        # ---- P4a: ypre[h*64+d, b] = sum_c wvT[c, h*64+d] * UT[c, b, h] ----
        # these ride the sync queue, which is idle after the last xa tile
        wvT_sb = consts.tile([128, NCHUNK, C], F32, tag="wvT_sb")
        nc.sync.dma_start(out=wvT_sb, in_=wvT.rearrange("(a p) o -> p a o", p=128))
        wpT_sb = consts.tile([128, NCHUNK, C], F32, tag="wpT_sb")
        nc.sync.dma_start(out=wpT_sb, in_=wpT.rearrange("(a p) o -> p a o", p=128))
        ypT_sb = consts.tile([128, NCHUNK, BSH], F32)
        for h in range(H):
            ps_yp = ps_misc.tile([64, BSH], F32, tag="misc")
            for k in range(NCHUNK):
                nc.tensor.matmul(
                    ps_yp,
                    lhsT=wvT_sb[:, k, h * 64:(h + 1) * 64],
                    rhs=ut_all[:, k, :, h],
                    start=(k == 0), stop=(k == NCHUNK - 1),
                )
            po = (h % 2) * 64
            nc.vector.tensor_copy(ypT_sb[po:po + 64, h // 2, :], ps_yp)

        # ---- P4b: y[b, c_out] = sum_c ypT[c, b] * wpT[c, c_out] + bp ----
        y_sb = small.tile([BSH, C], F32, tag="y")
        for j in range(2):
            ps_y = ps_misc.tile([BSH, 384], F32, tag="misc")
            for k in range(NCHUNK):
                nc.tensor.matmul(
                    ps_y,
                    lhsT=ypT_sb[:, k, :],
                    rhs=wpT_sb[:, k, j * 384:(j + 1) * 384],
                    start=(k == 0), stop=(k == NCHUNK - 1),
                )
            nc.vector.tensor_add(
                out=y_sb[:, j * 384:(j + 1) * 384],
                in0=ps_y,
                in1=bp_sb[:, j * 384:(j + 1) * 384],
            )
        nc.sync.dma_start(out=y, in_=y_sb)


_CACHE = {}


def kernel(x, wq, wk, wv, wp, bp, trace=False):
    x = np.ascontiguousarray(x, dtype=np.float32)
    wq = np.asarray(wq, dtype=np.float32)
    wk = np.asarray(wk, dtype=np.float32)
    wv = np.asarray(wv, dtype=np.float32)
    wp = np.asarray(wp, dtype=np.float32)
    bp = np.asarray(bp, dtype=np.float32)

    if "nc" not in _CACHE:
        _CACHE["nc"] = build_kernel()
    nc = _CACHE["nc"]

    x_sh = x.reshape(NCORES, BSH, N, C)
    wqT = np.ascontiguousarray(wq.T)
    wkn = np.ascontiguousarray(wk)
    wvT = np.ascontiguousarray(wv.T)
    wpT = np.ascontiguousarray(wp.T)
    bp2 = np.ascontiguousarray(bp.reshape(1, C))
    i12 = np.eye(H, dtype=np.float32)

    in_maps = []
    for k in range(NCORES):
        xs = x_sh[k]
        in_maps.append({
            "xT": np.ascontiguousarray(xs.transpose(0, 2, 1)),
            "x": np.ascontiguousarray(xs),
            "x0T": np.ascontiguousarray((xs[:, 0, :] * SCALE).T),
            "wqT": wqT,
            "wk": wkn,
            "wvT": wvT,
            "wpT": wpT,
            "bp": bp2,
            "i12": i12,
        })

    res = run_bass_kernel_spmd(nc, in_maps, core_ids=list(range(NCORES)),
                               trace=trace)
    out = np.concatenate([res.results[k]["y"] for k in range(NCORES)], axis=0)
    out = out.reshape(B, 1, C)
    if trace:
        _CACHE["last_exec_time_ns"] = res.exec_time_ns
        _CACHE["last_results"] = res
    return out



# revision 4
# speedup vs baseline: 2.4352x; 2.4352x over previous
"""Trainium2 Bass kernel for nn_CrossAttention (single-CLS-query cross attention).

Reference, per batch b:
    q = x[b,0,:] @ wq.T                   (single CLS query)
    k = x[b] @ wk.T ; v = x[b] @ wv.T
    out = softmax(q k^T / sqrt(d)) v ; y = out @ wp.T + bp

With one query token the K/V projections fold away algebraically:
    m[h,:]    = SCALE * q_h @ Wk_h                     [H, C]
    scoresT   = x[b] @ m.T                             [N, H]  (contract C)
    attnT     = exp(scoresT);  s_h = sum_n attnT[n,h]
    UT[c,h]   = sum_n x[b,n,c] attnT[n,h] / s_h        (contract N)
    y[b]      = concat_h(UT[:,h] @ Wv_h.T) @ wp.T + bp

Distribution: data parallel over batch B=32 across 8 cores (4/core), no
collectives.

Implementation: x is streamed from HBM ONCE per batch, bf16, in [C, N]
layout.  Phase A uses the x chunks as the matmul *stationary* operand with
the 12-wide m as moving operand (12-row matmuls).  Phase C needs x with N
on partitions, so every [128,128] chunk is transposed on the PE (bf16) and
copied PSUM->SBUF with the copies split across the DVE and Act engines;
phase C then again uses the transposed chunks as stationary with the
12-wide attnT moving.  Softmax denominators come from an all-ones matmul
(partition reduction), with normalization folded into the U scaling.
"""

import numpy as np
import ml_dtypes

import concourse.bass as bass
import concourse.tile as tile
from concourse import bacc, mybir
from concourse.bass_utils import run_bass_kernel_spmd

# Problem constants (hardcoded per the harness contract).
B, N, C = 32, 4096, 768
H, D = 12, 64
SCALE = D ** -0.5
NCORES = 8
BSH = B // NCORES   # batches per core
NCH = C // 128      # 6 c-chunks
NN = N // 128       # 32 n-chunks
NSL = N // 4        # DMA slice width along n
NG = 8              # transposed chunks per PSUM bank copy

F32 = mybir.dt.float32
BF16 = mybir.dt.bfloat16
BFNP = ml_dtypes.bfloat16


def build_kernel():
    nc = bacc.Bacc("TRN2", target_bir_lowering=False, debug=False,
                   num_devices=NCORES)

    xT = nc.dram_tensor("xT", [BSH, C, N], BF16, kind="ExternalInput")
    x0T = nc.dram_tensor("x0T", [C, BSH], BF16, kind="ExternalInput")
    wqT = nc.dram_tensor("wqT", [C, C], BF16, kind="ExternalInput")
    wk = nc.dram_tensor("wk", [C, C], BF16, kind="ExternalInput")
    wvT = nc.dram_tensor("wvT", [C, C], BF16, kind="ExternalInput")
    wpT = nc.dram_tensor("wpT", [C, C], BF16, kind="ExternalInput")
    bp = nc.dram_tensor("bp", [1, C], F32, kind="ExternalInput")
    ident = nc.dram_tensor("ident", [128, 128], BF16, kind="ExternalInput")
    ones = nc.dram_tensor("ones", [128, 128], BF16, kind="ExternalInput")
    y = nc.dram_tensor("y", [BSH, C], F32, kind="ExternalOutput")

    with tile.TileContext(nc) as tc:
        cross_attn_kernel(tc, y.ap(), xT.ap(), x0T.ap(), wqT.ap(), wk.ap(),
                          wvT.ap(), wpT.ap(), bp.ap(), ident.ap(), ones.ap())
    nc.compile()
    return nc


def cross_attn_kernel(tc, y, xT, x0T, wqT, wk, wvT, wpT, bp, ident, ones):
    from contextlib import ExitStack
    ctx = ExitStack()
    nc = tc.nc
    with ctx:
        consts = ctx.enter_context(tc.tile_pool(name="consts", bufs=1))
        xt_pool = ctx.enter_context(tc.tile_pool(name="xt", bufs=2))
        xnat_pool = ctx.enter_context(tc.tile_pool(name="xnat", bufs=1))
        attn_pool = ctx.enter_context(tc.tile_pool(name="attn", bufs=2))
        small = ctx.enter_context(tc.tile_pool(name="small", bufs=2))
        ps_s = ctx.enter_context(tc.tile_pool(name="ps_s", bufs=2, space="PSUM"))
        ps_t = ctx.enter_context(tc.tile_pool(name="ps_t", bufs=2, space="PSUM"))
        ps_u = ctx.enter_context(tc.tile_pool(name="ps_u", bufs=1, space="PSUM"))
        ps_misc = ctx.enter_context(tc.tile_pool(name="ps_misc", bufs=2, space="PSUM"))
        ps_sum = ctx.enter_context(tc.tile_pool(name="ps_sum", bufs=1, space="PSUM"))

        # ---- constant loads (scalar HWDGE queue; sync queue streams x) ----
        ident_sb = consts.tile([128, 128], BF16)
        nc.scalar.dma_start(out=ident_sb, in_=ident)
        ones_sb = consts.tile([128, 128], BF16)
        nc.scalar.dma_start(out=ones_sb, in_=ones)
        x0T_sb = consts.tile([128, NCH, BSH], BF16)
        nc.scalar.dma_start(out=x0T_sb, in_=x0T.rearrange("(a p) b -> p a b", p=128))
        wqT_sb = consts.tile([128, NCH, C], BF16, tag="wqT_sb")
        nc.scalar.dma_start(out=wqT_sb, in_=wqT.rearrange("(a p) o -> p a o", p=128))
        wk_sb = consts.tile([128, NCH, C], BF16, tag="wk_sb")
        nc.scalar.dma_start(out=wk_sb, in_=wk.rearrange("(a p) o -> p a o", p=128))
        bp_sb = consts.tile([BSH, C], F32)
        nc.scalar.dma_start(
            out=bp_sb,
            in_=bass.AP(tensor=bp.tensor, offset=0, ap=[[0, BSH], [1, C]]),
        )

        # ---- P0a: qT[c_out, b] = wq @ (SCALE * x0^T) ----
        qT_sb = consts.tile([128, NCH, BSH], BF16)
        for co in range(NCH):
            ps_q = ps_misc.tile([128, BSH], F32, tag="misc")
            for ci in range(NCH):
                nc.tensor.matmul(
                    ps_q,
                    lhsT=wqT_sb[:, ci, co * 128:(co + 1) * 128],
                    rhs=x0T_sb[:, ci, :],
                    start=(ci == 0), stop=(ci == NCH - 1),
                )
            nc.vector.tensor_copy(qT_sb[:, co, :], ps_q)

        # ---- P0b: mT[c, h, b] = Wk_h^T @ qT_h  (contraction over d=64) ----
        # NOTE: a matmul whose inputs sit at base partition 64 must write a
        # whole PSUM tile -- sub-slice outputs there fail BIR verification.
        mT_sb = consts.tile([128, NCH, H, BSH], BF16)
        for ci in range(NCH):
            for h in range(H):
                po = (h % 2) * 64
                ch = h // 2
                ps_m = ps_misc.tile([128, BSH], F32, tag="misc")
                nc.tensor.matmul(
                    ps_m,
                    lhsT=wk_sb[po:po + 64, ch, ci * 128:(ci + 1) * 128],
                    rhs=qT_sb[po:po + 64, ch, :],
                    start=True, stop=True,
                )
                nc.vector.tensor_copy(mT_sb[:, ci, h, :], ps_m)

        ut_all = consts.tile([128, NCH, H, BSH], BF16)  # U^T[c, h, b] normalized

        # ---- per-batch main loop ----
        for b in range(BSH):
            xT_sb = xt_pool.tile([128, NCH, N], BF16, tag="xT")
            for nt in range(4):
                nc.sync.dma_start(
                    out=xT_sb[:, :, nt * NSL:(nt + 1) * NSL],
                    in_=xT[b, :, nt * NSL:(nt + 1) * NSL].rearrange(
                        "(a p) n -> p a n", p=128),
                )

            # phase A: scoresT[n, h] accumulated over c-chunks, x stationary
            psS = ps_s.tile([128, NN, H], F32, tag="psS")
            for nn in range(NN):
                for ci in range(NCH):
                    nc.tensor.matmul(
                        psS[:, nn, :],
                        lhsT=xT_sb[:, ci, nn * 128:(nn + 1) * 128],
                        rhs=mT_sb[:, ci, :, b],
                        start=(ci == 0), stop=(ci == NCH - 1),
                    )
            # exp over the whole bank in one activation (|scores| < 8, no
            # max subtraction needed)
            attnT = attn_pool.tile([128, NN, H], BF16, tag="attnT")
            nc.scalar.activation(out=attnT, in_=psS,
                                 func=mybir.ActivationFunctionType.Exp)

            # softmax denominators: ones-matmul partition reduction
            # (replicated across partitions), then reduce over n-chunks
            psSum = ps_sum.tile([128, NN, H], F32, tag="psSum")
            nc.tensor.matmul(psSum, lhsT=ones_sb,
                             rhs=attnT.rearrange("p a h -> p (a h)"),
                             start=True, stop=True)
            sums_sb = small.tile([128, H], F32, tag="sums")
            nc.vector.reduce_sum(
                sums_sb,
                bass.AP(tensor=psSum.tensor, offset=psSum.offset,
                        ap=[psSum.ap[0], [1, H], [H, NN]]),
                axis=mybir.AxisListType.X,
            )
            rinv = small.tile([128, H], F32, tag="rinv")
            nc.vector.reciprocal(rinv, sums_sb)

            # transpose x chunks (ci-major, NG chunks per PSUM bank) with
            # PSUM->SBUF copies split across DVE and Act
            xnat = xnat_pool.tile([128, NCH, NN, 128], BF16, tag="xnat")
            cnt = 0
            for ci in range(NCH):
                for ng in range(NN // NG):
                    psT = ps_t.tile([128, NG, 128], BF16, tag="psT")
                    for k in range(NG):
                        nn = ng * NG + k
                        nc.tensor.transpose(
                            psT[:, k, :],
                            in_=xT_sb[:, ci, nn * 128:(nn + 1) * 128],
                            identity=ident_sb,
                        )
                    dst = xnat[:, ci, ng * NG:(ng + 1) * NG, :]
                    if (cnt % 5) < 3:
                        nc.vector.tensor_copy(out=dst, in_=psT)
                    else:
                        nc.scalar.copy(out=dst, in_=psT)
                    cnt += 1

            # phase C: UT[c, h] = sum_n x[n, c] attnT[n, h], xnat stationary;
            # one accumulation group per bank open at a time (ci-outer)
            psU = ps_u.tile([128, NCH, H], F32, tag="psU")
            for ci in range(NCH):
                for nn in range(NN):
                    nc.tensor.matmul(
                        psU[:, ci, :],
                        lhsT=xnat[:, ci, nn, :],
                        rhs=attnT[:, nn, :],
                        start=(nn == 0), stop=(nn == NN - 1),
                    )
            # normalize by softmax denominators while moving PSUM -> SBUF
            for ci in range(NCH):
                nc.vector.tensor_mul(
                    out=ut_all[:, ci, :, b], in0=psU[:, ci, :], in1=rinv)

            if b == 0:
                # late weights: only needed by P4, keep them off the head of
                # the DMA queue
                wvT_sb = consts.tile([128, NCH, C], BF16, tag="wvT_sb")
                nc.scalar.dma_start(
                    out=wvT_sb, in_=wvT.rearrange("(a p) o -> p a o", p=128))
                wpT_sb = consts.tile([128, NCH, C], BF16, tag="wpT_sb")
                nc.scalar.dma_start(
                    out=wpT_sb, in_=wpT.rearrange("(a p) o -> p a o", p=128))

        # ---- P4a: ypT[h*64+d, b] = sum_c wvT[c, h*64+d] * UT[c, h, b] ----
        ypT_sb = consts.tile([128, NCH, BSH], BF16)
        for h in range(H):
            ps_yp = ps_misc.tile([64, BSH], F32, tag="misc")
            for k in range(NCH):
                nc.tensor.matmul(
                    ps_yp,
                    lhsT=wvT_sb[:, k, h * 64:(h + 1) * 64],
                    rhs=ut_all[:, k, h, :],
                    start=(k == 0), stop=(k == NCH - 1),
                )
            po = (h % 2) * 64
            nc.vector.tensor_copy(ypT_sb[po:po + 64, h // 2, :], ps_yp)

        # ---- P4b: y[b, c_out] = sum_c ypT[c, b] * wpT[c, c_out] + bp ----
        y_sb = small.tile([BSH, C], F32, tag="y")
        for j in range(2):
            ps_y = ps_misc.tile([BSH, 384], F32, tag="misc")
            for k in range(NCH):
                nc.tensor.matmul(
                    ps_y,
                    lhsT=ypT_sb[:, k, :],
                    rhs=wpT_sb[:, k, j * 384:(j + 1) * 384],
                    start=(k == 0), stop=(k == NCH - 1),
                )
            nc.vector.tensor_add(
                out=y_sb[:, j * 384:(j + 1) * 384],
                in0=ps_y,
                in1=bp_sb[:, j * 384:(j + 1) * 384],
            )
        nc.sync.dma_start(out=y, in_=y_sb)


_CACHE = {}


def kernel(x, wq, wk, wv, wp, bp, trace=False):
    x = np.asarray(x, dtype=np.float32)
    wq = np.asarray(wq, dtype=np.float32)
    wk = np.asarray(wk, dtype=np.float32)
    wv = np.asarray(wv, dtype=np.float32)
    wp = np.asarray(wp, dtype=np.float32)
    bp = np.asarray(bp, dtype=np.float32)

    if "nc" not in _CACHE:
        _CACHE["nc"] = build_kernel()
    nc = _CACHE["nc"]

    x_sh = x.reshape(NCORES, BSH, N, C)
    wqT_bf = np.ascontiguousarray(wq.T).astype(BFNP)
    wk_bf = np.ascontiguousarray(wk).astype(BFNP)
    wvT_bf = np.ascontiguousarray(wv.T).astype(BFNP)
    wpT_bf = np.ascontiguousarray(wp.T).astype(BFNP)
    bp2 = np.ascontiguousarray(bp.reshape(1, C))
    ident = np.eye(128, dtype=BFNP)
    ones = np.ones((128, 128), dtype=BFNP)

    in_maps = []
    for k in range(NCORES):
        xs = x_sh[k]
        in_maps.append({
            "xT": xs.transpose(0, 2, 1).astype(BFNP),
            "x0T": np.ascontiguousarray((xs[:, 0, :] * SCALE).T).astype(BFNP),
            "wqT": wqT_bf,
            "wk": wk_bf,
            "wvT": wvT_bf,
            "wpT": wpT_bf,
            "bp": bp2,
            "ident": ident,
            "ones": ones,
        })

    res = run_bass_kernel_spmd(nc, in_maps, core_ids=list(range(NCORES)),
                               trace=trace)
    out = np.concatenate([res.results[k]["y"] for k in range(NCORES)], axis=0)
    out = out.reshape(B, 1, C)
    if trace:
        _CACHE["last_exec_time_ns"] = res.exec_time_ns
        _CACHE["last_results"] = res
    return out


# revision 5
# speedup vs baseline: 2.9985x; 1.2313x over previous
"""Trainium2 Bass kernel for nn_CrossAttention (single-CLS-query cross attention).

Reference, per batch b:
    q = x[b,0,:] @ wq.T                   (single CLS query)
    k = x[b] @ wk.T ; v = x[b] @ wv.T
    out = softmax(q k^T / sqrt(d)) v ; y = out @ wp.T + bp

With one query token the K/V projections fold away algebraically:
    m[h,:]    = SCALE * q_h @ Wk_h                     [H, C]
    scoresT   = x[b] @ m.T                             [N, H]  (contract C)
    attnT     = exp(scoresT);  s_h = sum_n attnT[n,h]
    UT[c,h]   = sum_n x[b,n,c] attnT[n,h] / s_h        (contract N)
    y[b]      = concat_h(UT[:,h] @ Wv_h.T) @ wp.T + bp

Distribution: data parallel over batch B=32 across 8 cores (4/core), no
collectives.

Implementation: x is streamed from HBM ONCE per batch, bf16, in [C, N]
layout.  Phase A uses the x chunks as the matmul *stationary* operand with
the 12-wide m as moving operand (12-row matmuls).  Phase C needs x with N
on partitions, so every [128,128] chunk is transposed on the PE (bf16) and
copied PSUM->SBUF with the copies split across the DVE and Act engines;
phase C then again uses the transposed chunks as stationary with the
12-wide attnT moving.  Softmax denominators come from an all-ones matmul
(partition reduction), with normalization folded into the U scaling.
"""

import numpy as np
import ml_dtypes

import concourse.bass as bass
import concourse.tile as tile
from concourse import bacc, mybir
from concourse.bass_utils import run_bass_kernel_spmd

# Problem constants (hardcoded per the harness contract).
B, N, C = 32, 4096, 768
H, D = 12, 64
SCALE = D ** -0.5
NCORES = 8
BSH = B // NCORES   # batches per core
NCH = C // 128      # 6 c-chunks
NN = N // 128       # 32 n-chunks
NSL = N // 4        # DMA slice width along n
NG = 8              # transposed chunks per PSUM bank copy

F32 = mybir.dt.float32
BF16 = mybir.dt.bfloat16
BFNP = ml_dtypes.bfloat16


def build_kernel():
    nc = bacc.Bacc("TRN2", target_bir_lowering=False, debug=False,
                   num_devices=NCORES)

    xT = nc.dram_tensor("xT", [BSH, C, N], BF16, kind="ExternalInput")
    x0T = nc.dram_tensor("x0T", [C, BSH], BF16, kind="ExternalInput")
    wqT = nc.dram_tensor("wqT", [C, C], BF16, kind="ExternalInput")
    wk = nc.dram_tensor("wk", [C, C], BF16, kind="ExternalInput")
    wvT = nc.dram_tensor("wvT", [C, C], BF16, kind="ExternalInput")
    wpT = nc.dram_tensor("wpT", [C, C], BF16, kind="ExternalInput")
    bp = nc.dram_tensor("bp", [1, C], F32, kind="ExternalInput")
    ident = nc.dram_tensor("ident", [128, 128], BF16, kind="ExternalInput")
    ones = nc.dram_tensor("ones", [128, 128], BF16, kind="ExternalInput")
    y = nc.dram_tensor("y", [BSH, C], F32, kind="ExternalOutput")

    with tile.TileContext(nc) as tc:
        cross_attn_kernel(tc, y.ap(), xT.ap(), x0T.ap(), wqT.ap(), wk.ap(),
                          wvT.ap(), wpT.ap(), bp.ap(), ident.ap(), ones.ap())
    nc.compile()
    return nc


def cross_attn_kernel(tc, y, xT, x0T, wqT, wk, wvT, wpT, bp, ident, ones):
    from contextlib import ExitStack
    ctx = ExitStack()
    nc = tc.nc
    with ctx:
        consts = ctx.enter_context(tc.tile_pool(name="consts", bufs=1))
        xt_pool = ctx.enter_context(tc.tile_pool(name="xt", bufs=2))
        xnat_pool = ctx.enter_context(tc.tile_pool(name="xnat", bufs=1))
        attn_pool = ctx.enter_context(tc.tile_pool(name="attn", bufs=2))
        small = ctx.enter_context(tc.tile_pool(name="small", bufs=2))
        ps_s = ctx.enter_context(tc.tile_pool(name="ps_s", bufs=1, space="PSUM"))
        ps_t = ctx.enter_context(tc.tile_pool(name="ps_t", bufs=3, space="PSUM"))
        ps_u = ctx.enter_context(tc.tile_pool(name="ps_u", bufs=1, space="PSUM"))
        ps_misc = ctx.enter_context(tc.tile_pool(name="ps_misc", bufs=2, space="PSUM"))
        ps_sum = ctx.enter_context(tc.tile_pool(name="ps_sum", bufs=1, space="PSUM"))

        # ---- constant loads (scalar HWDGE queue; sync queue streams x) ----
        # P0-critical tensors first so m is ready when the first x slice lands
        x0T_sb = consts.tile([128, NCH, BSH], BF16)
        nc.scalar.dma_start(out=x0T_sb, in_=x0T.rearrange("(a p) b -> p a b", p=128))
        wqT_sb = consts.tile([128, NCH, C], BF16, tag="wqT_sb")
        nc.scalar.dma_start(out=wqT_sb, in_=wqT.rearrange("(a p) o -> p a o", p=128))
        wk_sb = consts.tile([128, NCH, C], BF16, tag="wk_sb")
        nc.scalar.dma_start(out=wk_sb, in_=wk.rearrange("(a p) o -> p a o", p=128))
        ident_sb = consts.tile([128, 128], BF16)
        nc.scalar.dma_start(out=ident_sb, in_=ident)
        ones_sb = consts.tile([128, 128], BF16)
        nc.scalar.dma_start(out=ones_sb, in_=ones)
        bp_sb = consts.tile([BSH, C], F32)
        nc.scalar.dma_start(
            out=bp_sb,
            in_=bass.AP(tensor=bp.tensor, offset=0, ap=[[0, BSH], [1, C]]),
        )

        # ---- P0a: qT[c_out, b] = wq @ (SCALE * x0^T) ----
        qT_sb = consts.tile([128, NCH, BSH], BF16)
        for co in range(NCH):
            ps_q = ps_misc.tile([128, BSH], F32, tag="misc")
            for ci in range(NCH):
                nc.tensor.matmul(
                    ps_q,
                    lhsT=wqT_sb[:, ci, co * 128:(co + 1) * 128],
                    rhs=x0T_sb[:, ci, :],
                    start=(ci == 0), stop=(ci == NCH - 1),
                )
            nc.vector.tensor_copy(qT_sb[:, co, :], ps_q)

        # ---- P0b: mT[c, h, b] = Wk_h^T @ qT_h  (contraction over d=64) ----
        # NOTE: a matmul whose inputs sit at base partition 64 must write a
        # whole PSUM tile -- sub-slice outputs there fail BIR verification.
        mT_sb = consts.tile([128, NCH, H, BSH], BF16)
        for ci in range(NCH):
            for h in range(H):
                po = (h % 2) * 64
                ch = h // 2
                ps_m = ps_misc.tile([128, BSH], F32, tag="misc")
                nc.tensor.matmul(
                    ps_m,
                    lhsT=wk_sb[po:po + 64, ch, ci * 128:(ci + 1) * 128],
                    rhs=qT_sb[po:po + 64, ch, :],
                    start=True, stop=True,
                )
                nc.vector.tensor_copy(mT_sb[:, ci, h, :], ps_m)

        ut_all = consts.tile([128, NCH, H, BSH], BF16)  # U^T[c, h, b] normalized

        # ---- per-batch main loop ----
        # Per DMA slice q (8 n-chunks): phase A -> exp -> transposes ->
        # partial phase C, so the transpose/copyback pipeline and phase C
        # run while the next slice streams in.  Copybacks alternate strictly
        # DVE/Act so both engines drain transpose banks concurrently.
        NQ = NN // NG  # 4 slices
        ypT_sb = consts.tile([128, NCH, BSH], BF16)
        cnt = 0

        def p4a(b):
            # ypre^T[h*64+d, b] = sum_c wvT[c, h*64+d] UT[c, h, b]
            for h in range(H):
                ps_yp = ps_misc.tile([64, 1], F32, tag="misc")
                for k in range(NCH):
                    nc.tensor.matmul(
                        ps_yp,
                        lhsT=wvT_sb[:, k, h * 64:(h + 1) * 64],
                        rhs=ut_all[:, k, h, b:b + 1],
                        start=(k == 0), stop=(k == NCH - 1),
                    )
                po = (h % 2) * 64
                nc.vector.tensor_copy(ypT_sb[po:po + 64, h // 2, b:b + 1], ps_yp)

        for b in range(BSH):
            if b == 2:
                p4a(0)
            elif b == 3:
                p4a(1)
                p4a(2)
            xT_sb = xt_pool.tile([128, NCH, N], BF16, tag="xT")
            psS = ps_s.tile([128, NN, H], F32, tag="psS")
            attnT = attn_pool.tile([128, NN, H], BF16, tag="attnT")
            xnat = xnat_pool.tile([128, NCH, NN, 128], BF16, tag="xnat")
            psU4 = ps_u.tile([128, NQ, NCH, H], F32, tag="psU")
            for q in range(NQ):
                nc.sync.dma_start(
                    out=xT_sb[:, :, q * NSL:(q + 1) * NSL],
                    in_=xT[b, :, q * NSL:(q + 1) * NSL].rearrange(
                        "(a p) n -> p a n", p=128),
                )
                # phase A on this slice: scoresT[n, h], x chunks stationary
                for nn in range(q * NG, (q + 1) * NG):
                    for ci in range(NCH):
                        nc.tensor.matmul(
                            psS[:, nn, :],
                            lhsT=xT_sb[:, ci, nn * 128:(nn + 1) * 128],
                            rhs=mT_sb[:, ci, :, b],
                            start=(ci == 0), stop=(ci == NCH - 1),
                        )
                nc.scalar.activation(
                    out=attnT[:, q * NG:(q + 1) * NG, :],
                    in_=psS[:, q * NG:(q + 1) * NG, :],
                    func=mybir.ActivationFunctionType.Exp)
                # transpose this slice's chunks (ci-major, one PSUM bank per
                # 8-chunk group)
                for ci in range(NCH):
                    psT = ps_t.tile([128, NG, 128], BF16, tag="psT")
                    for k in range(NG):
                        nn = q * NG + k
                        nc.tensor.transpose(
                            psT[:, k, :],
                            in_=xT_sb[:, ci, nn * 128:(nn + 1) * 128],
                            identity=ident_sb,
                        )
                    dst = xnat[:, ci, q * NG:(q + 1) * NG, :]
                    if cnt % 2 == 0:
                        nc.vector.tensor_copy(out=dst, in_=psT)
                    else:
                        nc.scalar.copy(out=dst, in_=psT)
                    cnt += 1
                # partial phase C for this slice (one group per bank at a time)
                for ci in range(NCH):
                    for k in range(NG):
                        nn = q * NG + k
                        nc.tensor.matmul(
                            psU4[:, q, ci, :],
                            lhsT=xnat[:, ci, nn, :],
                            rhs=attnT[:, nn, :],
                            start=(k == 0), stop=(k == NG - 1),
                        )

            # softmax denominators: ones-matmul partition reduction
            # (replicated across partitions), then reduce over n-chunks
            psSum = ps_sum.tile([128, NN, H], F32, tag="psSum")
            nc.tensor.matmul(psSum, lhsT=ones_sb,
                             rhs=attnT.rearrange("p a h -> p (a h)"),
                             start=True, stop=True)
            sums_sb = small.tile([128, H], F32, tag="sums")
            nc.vector.reduce_sum(
                sums_sb,
                bass.AP(tensor=psSum.tensor, offset=psSum.offset,
                        ap=[psSum.ap[0], [1, H], [H, NN]]),
                axis=mybir.AxisListType.X,
            )
            rinv = small.tile([128, H], F32, tag="rinv")
            nc.vector.reciprocal(rinv, sums_sb)

            # combine the 4 slice-partials of UT, then normalize into ut_all
            uq_sb = small.tile([128, NCH, H], F32, tag="uq")
            nc.vector.reduce_sum(
                uq_sb,
                bass.AP(tensor=psU4.tensor, offset=psU4.offset,
                        ap=[psU4.ap[0], [1, NCH * H], [NCH * H, NQ]]),
                axis=mybir.AxisListType.X,
            )
            for ci in range(NCH):
                nc.vector.tensor_mul(
                    out=ut_all[:, ci, :, b], in0=uq_sb[:, ci, :], in1=rinv)

            if b == 0:
                # late weights: only needed by P4, keep them off the head of
                # the DMA queue
                wvT_sb = consts.tile([128, NCH, C], BF16, tag="wvT_sb")
                nc.scalar.dma_start(
                    out=wvT_sb, in_=wvT.rearrange("(a p) o -> p a o", p=128))
                wpT_sb = consts.tile([128, NCH, C], BF16, tag="wpT_sb")
                nc.scalar.dma_start(
                    out=wpT_sb, in_=wpT.rearrange("(a p) o -> p a o", p=128))
        p4a(3)

        # ---- P4b: y[b, c_out] = sum_c ypT[c, b] * wpT[c, c_out] + bp ----
        y_sb = small.tile([BSH, C], F32, tag="y")
        for j in range(2):
            ps_y = ps_misc.tile([BSH, 384], F32, tag="misc")
            for k in range(NCH):
                nc.tensor.matmul(
                    ps_y,
                    lhsT=ypT_sb[:, k, :],
                    rhs=wpT_sb[:, k, j * 384:(j + 1) * 384],
                    start=(k == 0), stop=(k == NCH - 1),
                )
            nc.vector.tensor_add(
                out=y_sb[:, j * 384:(j + 1) * 384],
                in0=ps_y,
                in1=bp_sb[:, j * 384:(j + 1) * 384],
            )
        nc.sync.dma_start(out=y, in_=y_sb)


_CACHE = {}


def kernel(x, wq, wk, wv, wp, bp, trace=False):
    x = np.asarray(x, dtype=np.float32)
    wq = np.asarray(wq, dtype=np.float32)
    wk = np.asarray(wk, dtype=np.float32)
    wv = np.asarray(wv, dtype=np.float32)
    wp = np.asarray(wp, dtype=np.float32)
    bp = np.asarray(bp, dtype=np.float32)

    if "nc" not in _CACHE:
        _CACHE["nc"] = build_kernel()
    nc = _CACHE["nc"]

    x_sh = x.reshape(NCORES, BSH, N, C)
    wqT_bf = np.ascontiguousarray(wq.T).astype(BFNP)
    wk_bf = np.ascontiguousarray(wk).astype(BFNP)
    wvT_bf = np.ascontiguousarray(wv.T).astype(BFNP)
    wpT_bf = np.ascontiguousarray(wp.T).astype(BFNP)
    bp2 = np.ascontiguousarray(bp.reshape(1, C))
    ident = np.eye(128, dtype=BFNP)
    ones = np.ones((128, 128), dtype=BFNP)

    in_maps = []
    for k in range(NCORES):
        xs = x_sh[k]
        in_maps.append({
            "xT": xs.transpose(0, 2, 1).astype(BFNP),
            "x0T": np.ascontiguousarray((xs[:, 0, :] * SCALE).T).astype(BFNP),
            "wqT": wqT_bf,
            "wk": wk_bf,
            "wvT": wvT_bf,
            "wpT": wpT_bf,
            "bp": bp2,
            "ident": ident,
            "ones": ones,
        })

    res = run_bass_kernel_spmd(nc, in_maps, core_ids=list(range(NCORES)),
                               trace=trace)
    out = np.concatenate([res.results[k]["y"] for k in range(NCORES)], axis=0)
    out = out.reshape(B, 1, C)
    if trace:
        _CACHE["last_exec_time_ns"] = res.exec_time_ns
        _CACHE["last_results"] = res
    return out
